# revision 26
# baseline (speedup 1.0000x reference)
"""Trainium2 Bass kernel for BF16IndexerBaseline (sparse_attention).

Computes, for q:(1,M,H,D) bf16, k:(1,N,D) bf16, weights:(H,M) bf16:

    index_score[b,m,n] = sum_h relu(q[b,m,h,:] . k[b,n,:]) * (weights[h,m]*D**-0.5)

Strategy (8 NeuronCores, SPMD, host-side sharding of m):
  - each core gets an m-shard of 256 rows (2 m-tiles of 128), k replicated.
  - weights >= 0, so the per-(m,h) scale commutes with relu and is folded
    into q ON THE HOST: q' = q * (w*scale) -> the device computes plain
    sum_h relu(q'.k). Both q' and k are pre-transposed on the host into
    [D=128, cols] layout, so the kernel has NO device-side transposes, no
    weights load, and no per-partition scale vectors.
  - per (m-tile, n-chunk of 1024) unit: 16 heads x 2 matmuls (K=D=128,
    stationary qT block, moving kT) -> fp32 logits in PSUM [128,1024]
    tiles (2-buf psA pool for ACT-heads, 2-buf psV for DVE-heads).
  - eviction split (PSUM reads: 1 elem/lane/cyc fp32, ACT+DVE only):
      * 9 "A" heads on ScalarE: relu -> bf16 leaves (~1.20us/tile).
      * 7 "V" heads on VectorE via the fused custom DVE op
        RELU_SCALE_ADD: acc = relu(psum) + in1 (~1.28us/tile, the
        accumulate is free). c0 seeds from leaf r0 (evicted 2 slots
        earlier); the last chain op folds acc and writes the bf16 stage.
      * leaf tree kept OFF VectorE: 4 in-place SWDGE ring adds + 2
        GpSimd adds per unit; only the final two bf16 merges (x=r1+t,
        stage+=x, ~0.68us each) run on VectorE, in the next unit's slack.
  - output bf16 (halves out-DMA traffic; host casts to fp32).
  - last unit: no ring/GpSimd ops on the critical tail; trailing heads
    split across both engines and the tree finishes on VectorE.
  - PE warm-up burst at t=0 trips the HAM clock gate to 2.4 GHz; a dummy
    ACTIVATE preloads the relu table set.
"""

import os

os.environ.setdefault("MYCRO_LOCAL_CACHE", "1")

import numpy as np
import ml_dtypes
from contextlib import ExitStack

import concourse.bass as bass
import concourse.tile as tile
from concourse import bacc, mybir
from concourse.bass_utils import run_bass_kernel_spmd

# ---------------------------------------------------------------- problem dims
B = 1
M = 2048
H = 16
N = 4096
D = 128
N_CORES = 8
MS = M // N_CORES          # 256 rows of m per core
MT = MS // 128             # 2 m-tiles per core
FD = 1024                  # n-chunk (free dim) per eviction op = 2 PSUM banks
NCH = N // FD              # 4 n-chunks
WARMUP_MMS = int(os.environ.get("IDX_WARMUP_MMS", "5"))
MM_W = 512                 # matmul moving width (1 PSUM bank fp32)

# steady-unit role string: A = ScalarE relu leaf, V = VectorE chain head.
ROLES = os.environ.get("IDX_ROLES", "AVAVAVAVAVAVAVAA")
ROLES_LAST = os.environ.get("IDX_ROLES_LAST", "AVAVAVAVAVAVAVAD")
# D = DVE TS eviction (leaf evicted on VectorE; used near the tail)

BF16 = mybir.dt.bfloat16
F32 = mybir.dt.float32
SCALE_BF16 = float(np.float32(np.array(D ** -0.5, dtype=ml_dtypes.bfloat16)))

# --------------------------------------------------- custom fused DVE op
# out = relu(in0 * s0) + in1   (s0 per-partition scalar [P,1]; used with ones)
import concourse.dve_ops as dve_ops
from concourse.dve_spec import Spec as _Spec, Src0 as _Src0, Src1 as _Src1, C0 as _C0
from concourse.dve_spec import relu as _relu, lower as _lower
from concourse.dve_uop import DveOpSpec as _DveOpSpec

_OP_NAME = "RELU_SCALE_ADD_ANT"


def _ref_relu_scale_add(in0, in1, s0, s1, imm2):
    x = np.nan_to_num(in0.astype(np.float32) * s0, nan=0.0, posinf=np.inf, neginf=-np.inf)
    return np.maximum(x, 0.0).astype(np.float32) + in1


def _register_relu_scale_add():
    for op in dve_ops.OPS:
        if op.name == _OP_NAME:
            return op
    spec = _Spec(body=_relu(_Src0 * _C0) + _Src1, reference=_ref_relu_scale_add)
    row = max(dve_ops._SUB_OPCODE_FOR_NAME.values()) + 1
    assert row < 0x20
    dve_ops._SUB_OPCODE_FOR_NAME[_OP_NAME] = row
    shas = {
        v: _DveOpSpec(name=_OP_NAME, opcode=row, uops=_lower(spec, ver=v), rd1_en=True).sha(v)
        for v in ("v3", "v4")
    }
    op = dve_ops.DveOp(_OP_NAME, spec, subdim=False, uops_sha=shas)
    dve_ops.OPS.append(op)
    dve_ops.CUSTOM_DVE_SPECS[_OP_NAME] = spec
    return op


RELU_SCALE_ADD = _register_relu_scale_add()


# ------------------------------------------------------------------ kernel IR
def _emit(ctx: ExitStack, tc: "tile.TileContext", q_d, k_d, o_d):
    nc = tc.nc
    AOp = mybir.AluOpType

    const = ctx.enter_context(tc.tile_pool(name="const", bufs=1))
    psA = ctx.enter_context(tc.tile_pool(name="psA", bufs=2, space="PSUM"))
    psV = ctx.enter_context(tc.tile_pool(name="psV", bufs=2, space="PSUM"))
    rpool = ctx.enter_context(tc.tile_pool(name="rpool", bufs=5))
    tpool = ctx.enter_context(tc.tile_pool(name="tpool", bufs=4))
    apool = ctx.enter_context(tc.tile_pool(name="apool", bufs=4))
    opool = ctx.enter_context(tc.tile_pool(name="opool", bufs=8))

    # ---- t=0 dummies: bf16 zero tile (VectorE memset) feeds a warm-up MM
    # burst (HAM -> 2.4 GHz); a 1-col ACTIVATE preloads the relu table set.
    dummy = const.tile([128, 512], BF16)
    nc.vector.memset(dummy[:], 0.0)
    if WARMUP_MMS:
        wu_ps = psA.tile([128, FD], F32, tag="logits", name="wu_ps")
        for i in range(WARMUP_MMS):
            nc.tensor.matmul(
                wu_ps[:, 0:512], dummy[:, 0:128], dummy[:], start=True, stop=True
            )
    # ---- input loads (plain 2D DMA, no transposes): pieces ordered so unit
    # 0's data lands first, alternating across the sync + scalar HWDGE rings.
    # The relu-table preload (d_act) sits after the first kT piece so the
    # ~2.7us ACT_TABLE_LOAD overlaps the remaining loads.
    kT = const.tile([128, N], BF16)
    qT = const.tile([128, H * MS], BF16)          # col = mt*H*128 + h*128 + m
    QP = H * 128                                  # one m-tile's worth of q cols
    nc.scalar.dma_start(out=kT[:, 0:FD], in_=k_d[:, 0:FD])
    nc.sync.dma_start(out=qT[:, 0:512], in_=q_d[:, 0:512])
    d_act = const.tile([128, 1], BF16)
    nc.scalar.activation(d_act[:], dummy[:, 0:1], mybir.ActivationFunctionType.Relu)
    nc.sync.dma_start(out=qT[:, 512:QP], in_=q_d[:, 512:QP])
    nc.scalar.dma_start(out=kT[:, FD:2 * FD], in_=k_d[:, FD:2 * FD])
    nc.sync.dma_start(out=kT[:, 2 * FD:N], in_=k_d[:, 2 * FD:N])
    nc.sync.dma_start(out=qT[:, QP:2 * QP], in_=q_d[:, QP:2 * QP])

    # ones vector for the custom op's per-partition scale
    ones = const.tile([128, 1], F32)
    nc.vector.memset(ones[:], 1.0)

    n_units = MT * NCH
    # deferred-emission table: sched[(unit, slot)] -> [closures]. Tree
    # stages are emitted 1-2 units after their inputs so the issuing queue
    # (GpSimd FIFO for SWDGE/TT, Vector FIFO for the final fold) never
    # blocks on an in-flight dependency — a blocked queue convoys the
    # strictly-ordered matmul stream and starves both evictor engines.
    sched = {}

    def _at(u, s, fn):
        sched.setdefault((u, s), []).append(fn)

    for u in range(n_units):
        mt, nci = divmod(u, NCH)
        n0 = nci * FD
        uid = f"{mt}_{nci}"
        last = u == n_units - 1
        fastlag = u >= n_units - 3 and not last
        roles = ROLES_LAST if last else ROLES

        acc = apool.tile([128, FD], F32, tag="acc", name=f"acc_{uid}")
        stage = opool.tile([128, FD], BF16, tag="stage", name=f"stage_{uid}")
        # all 9 ACT leaves live in ONE contiguous block so pair-adds and
        # merges batch into single strided SWDGE DMAs (the ~1.1us SWDGE
        # issue cost on the GpSimd queue is per instruction, not per byte).
        lb = rpool.tile([128, 9 * FD], BF16, tag="lb", name=f"lb_{uid}")
        leaves = [lb[:, i * FD:(i + 1) * FD] for i in range(9)]
        n_leaf = 0
        n_v = roles.count("V")
        vi = 0              # chain index

        def _mk_head(h, pool, uid=uid, n0=n0, mt=mt):
            pt = pool.tile([128, FD], F32, tag="logits", name=f"ps_{uid}_{h}")
            lhs = qT[:, mt * QP + h * 128: mt * QP + h * 128 + 128]
            for j in range(FD // MM_W):
                nc.tensor.matmul(
                    pt[:, j * MM_W: (j + 1) * MM_W],
                    lhs,
                    kT[:, n0 + j * MM_W: n0 + (j + 1) * MM_W],
                    start=True,
                    stop=True,
                )
            return pt

        def _batch_pairs4(lb=lb):
            # one SWDGE DMA: (w0+=w1, w2+=w3, w4+=w5, w6+=w7)
            v4 = lb[:, 0:8 * FD].rearrange(
                "p (w two c) -> p w two c", two=2, c=FD
            )
            nc.gpsimd.dma_start(
                out=v4[:, :, 0, :], in_=v4[:, :, 1, :], accum_op=AOp.add
            )

        def _batch_merge(lb=lb):
            # one SWDGE DMA: (w0 += w2, w4 += w6)
            v8 = lb[:, 0:8 * FD].rearrange(
                "p (a b c) -> p a b c", a=2, c=2 * FD
            )
            nc.gpsimd.dma_start(
                out=v8[:, :, 0, 0:FD], in_=v8[:, :, 1, 0:FD], accum_op=AOp.add
            )

        def _ring(dst, src, lb=lb):
            nc.gpsimd.dma_start(
                out=lb[:, dst * FD:(dst + 1) * FD],
                in_=lb[:, src * FD:(src + 1) * FD],
                accum_op=AOp.add,
            )

        def _fold8(leaves=leaves):
            # GpSimd: w0 += w8 (w0 already holds r0..r7 via the rings)
            nc.gpsimd.tensor_add(leaves[0], leaves[0], leaves[8])

        def _finish(stage=stage, leaves=leaves, mt=mt, n0=n0):
            nc.vector.tensor_add(stage[:], stage[:], leaves[0])
            nc.sync.dma_start(
                out=o_d[mt * 128: (mt + 1) * 128, n0: n0 + FD], in_=stage[:]
            )

        for h, role in enumerate(roles):
            for fn in sched.pop((u, h), []):
                fn()
            if role == "A" or role == "D":
                pt = _mk_head(h, psA if role == "A" else psV)
                r = leaves[n_leaf]
                if role == "A":
                    nc.scalar.activation(
                        r, pt[:], mybir.ActivationFunctionType.Relu
                    )
                else:
                    nc.vector.tensor_scalar(
                        r, pt[:], 1.0, 0.0, op0=AOp.mult, op1=AOp.max
                    )
                li = n_leaf
                n_leaf += 1
                if last:
                    pass
                elif fastlag:
                    # last two pipelined units: eager single rings so every
                    # partial completes early and the finisher fits inside
                    # the next unit rather than after the whole kernel.
                    if li == 1:
                        _ring(0, 1)
                    elif li == 3:
                        _ring(2, 3)
                    elif li == 5:
                        _ring(0, 2)
                        _ring(4, 5)
                    elif li == 7:
                        _ring(0, 4)
                    elif li == 8:
                        _ring(6, 7)
                        _at(u + 1, 2, lambda f=_ring: f(0, 6))
                        _at(u + 1, 6, _fold8)
                        _at(u + 1, 12, _finish)
                elif li == 8:
                    # deep-lag pipeline: each stage runs >=1.5 units after
                    # its producer (ring DMA issue->completion is ~5us), so
                    # no issuing queue ever blocks on an in-flight stage.
                    _at(u + 1, 2, _batch_pairs4)
                    _at(u + 2, 2, _batch_merge)
                    _at(u + 2, 10, lambda f=_ring: f(0, 4))
                    _at(u + 3, 2, _fold8)
                    _at(u + 3, 8, _finish)
            else:  # V chain head
                pt = _mk_head(h, psV)
                if vi == 0:
                    nc.vector.tensor_scalar(
                        acc[:], pt[:], 1.0, 0.0, op0=AOp.mult, op1=AOp.max
                    )
                elif vi == n_v - 1:
                    nc.vector._custom_dve(
                        RELU_SCALE_ADD, out=stage[:], in0=pt[:],
                        in1=acc[:], s0=ones[:, 0:1],
                    )
                else:
                    nc.vector._custom_dve(
                        RELU_SCALE_ADD, out=acc[:], in0=pt[:],
                        in1=acc[:], s0=ones[:, 0:1],
                    )
                vi += 1

        if last:
            for key in sorted(k for k in list(sched) if k[0] <= u):
                for fn in sched.pop(key):
                    fn()
            # tail tree on VectorE/GpSimd only: GpSimd handles the early
            # leaves with slack; VectorE folds the late ones right after
            # their evictions so the post-matmul critical path is short.
            t1 = tpool.tile([128, FD], BF16, tag="t", name=f"t1_{uid}")
            nc.gpsimd.tensor_add(t1[:], leaves[0], leaves[1])
            nc.gpsimd.tensor_add(t1[:], t1[:], leaves[2])
            nc.gpsimd.tensor_add(t1[:], t1[:], leaves[3])
            x = tpool.tile([128, FD], BF16, tag="x", name=f"x_{uid}")
            nc.vector.tensor_add(x[:], leaves[4], leaves[5])
            nc.vector.tensor_add(x[:], x[:], t1[:])
            nc.vector.tensor_add(x[:], x[:], leaves[6])
            nc.vector.tensor_add(x[:], x[:], leaves[7])
            nc.vector.tensor_add(x[:], x[:], leaves[8])
            nc.vector.tensor_add(stage[:], stage[:], x[:])
            nc.sync.dma_start(
                out=o_d[mt * 128: (mt + 1) * 128, n0: n0 + FD], in_=stage[:]
            )


_NC_CACHE = None


def _build():
    global _NC_CACHE
    if _NC_CACHE is not None:
        return _NC_CACHE
    nc = bacc.Bacc(
        "TRN2",
        target_bir_lowering=False,
        debug=False,
        enable_asserts=False,
        num_devices=N_CORES,
    )
    q_d = nc.dram_tensor("qT", [D, H * MS], BF16, kind="ExternalInput").ap()
    k_d = nc.dram_tensor("kT", [D, N], BF16, kind="ExternalInput").ap()
    o_d = nc.dram_tensor("o", [MS, N], BF16, kind="ExternalOutput").ap()
    with tile.TileContext(nc) as tc:
        with ExitStack() as ctx:
            _emit(ctx, tc, q_d, k_d, o_d)
    nc.compile()
    _NC_CACHE = (nc, q_d, k_d, o_d)
    return _NC_CACHE


def _shard_inputs(q, k, weights):
    bf16 = ml_dtypes.bfloat16
    q = np.asarray(q).astype(bf16, copy=False).reshape(M, H, D)
    k = np.asarray(k).astype(bf16, copy=False).reshape(N, D)
    w = np.asarray(weights).astype(bf16, copy=False).reshape(H, M)
    # q_s matches the reference's bf16 rounding: bf16(w) * bf16(scale) -> bf16
    q_s = (w.astype(np.float32) * np.float32(SCALE_BF16)).astype(bf16)
    # pre-scale q (weights >= 0 so the scale commutes with relu)
    q_scaled = (q.astype(np.float32) * q_s.T[:, :, None].astype(np.float32)).astype(bf16)
    kT = np.ascontiguousarray(k.T)                      # [D, N]
    in_maps = []
    for c in range(N_CORES):
        m0 = c * MS
        # cols ordered m-tile-major: col = mt*H*128 + h*128 + m_local
        q_c = q_scaled[m0: m0 + MS].reshape(MT, 128, H, D).transpose(0, 2, 1, 3)
        qT_c = np.ascontiguousarray(
            q_c.reshape(MT * H * 128, D).T                # [D, MT*H*128]
        )
        in_maps.append({"qT": qT_c, "kT": kT})
    return in_maps


LAST_RESULTS = None


def kernel(q, k, weights):
    global LAST_RESULTS
    nc, *_ = _build()
    in_maps = _shard_inputs(q, k, weights)
    trace = bool(int(os.environ.get("IDX_TRACE", "0")))
    res = run_bass_kernel_spmd(
        nc, in_maps, core_ids=list(range(N_CORES)), trace=trace
    )
    LAST_RESULTS = res
    out = np.empty((B, M, N), np.float32)
    for c in range(N_CORES):
        out[0, c * MS: (c + 1) * MS] = res.results[c]["o"].astype(np.float32)
    return out


# revision 33
# speedup vs baseline: 1.3027x; 1.3027x over previous
"""Trainium2 Bass kernel for BF16IndexerBaseline (sparse_attention).

Computes, for q:(1,M,H,D) bf16, k:(1,N,D) bf16, weights:(H,M) bf16:

    index_score[b,m,n] = sum_h relu(q[b,m,h,:] . k[b,n,:]) * (weights[h,m]*D**-0.5)

Strategy (8 NeuronCores, SPMD, host-side sharding of m):
  - each core gets m-shard of 256 rows (2 m-tiles of 128), k replicated.
  - since weights >= 0, the per-(m,h) scale commutes with relu and is folded
    into the PSUM-eviction ops as a per-partition scalar (logits come out of
    the PE with m on partitions).
  - per (m-tile, n-chunk of 1024) unit: 16 heads x 2 matmuls (K=D=128
    contraction, stationary qT tile, moving kT) -> fp32 logits in PSUM
    ([128,1024] tiles, separate 2-buf pools for the A- and V-head roles).
  - epilogue split across engines (PSUM reads are the hard bottleneck:
    1 elem/lane/cyc per engine, ACT+DVE only):
      * 6 "chain" heads on VectorE via a runtime-registered fused custom
        DVE op RELU_SCALE_ADD: acc = relu(psum*s) + acc (fp32, 1 op/elem).
        The chain is kept independent of the ACT stream (no cross-seeding
        - coupling the two streams convoys the whole pipeline).
      * 10 heads on ScalarE: r = relu(psum*s) -> bf16 tiles; pair-summed
        as they land: 3 pairs on the DMA rings (SWDGE CCE accumulate,
        in-place SBUF->SBUF, no serial folds), 2 on GpSimd; balanced
        upper merge + final combine acc+root on VectorE (bf16 2x).
      * last unit: chain heads emitted first and late pairs on GpSimd /
        VectorE instead of the rings, so no chain overhang or DMA
        completion latency serializes into the kernel tail.
  - PE warm-up: a burst of dummy matmuls at t=0 keeps the PE HAM activity
    monitor busy through the input-transpose phase so real matmuls run at
    2.4 GHz instead of the cold 1.2 GHz.
  - startup: q sharded m-tile-major so unit 0 needs only the first q piece;
    q/k transposed through the DMA xbar in small pieces alternating across
    the two HWDGE rings (sync + scalar), critical pieces first; a dummy
    ACTIVATE preloads the relu table set before the transposes finish.
  - final: out = chain_acc + tree_root (fp32) -> DMA to DRAM.

Measured on 8x trn2 (NTFF profile): 124.7-126.0 us across runs in the
device's normal clock state (baseline 133.1 us); ~140-150 us when the
chip is in its P0 power-derate state (all engines ~15-20% slower —
device-side, affects any kernel equally). The
structure is pinned by hard limits: PSUM (16 KB/partition) holds exactly
4x [128,1024] fp32 tiles (2-buf ping-pong for each of the A/V roles, MM
output <= 1 bank = 512 fp32); eviction floor = 16.8M fp32 PSUM reads
through ACT (1.2 GHz) + DVE (0.96 GHz) at 1 elem/lane/cyc ~= 61 us + per
-op overheads (~(FD+282)/1.2 ns ACT, ~(FD+207)/0.96 ns DVE) ~= 87 us;
plus ~6.5 us framework preamble and ~6 us teardown barriers.
"""

import os

os.environ.setdefault("MYCRO_LOCAL_CACHE", "1")

import numpy as np
import ml_dtypes
from contextlib import ExitStack

import concourse.bass as bass
import concourse.tile as tile
from concourse import bacc, mybir
from concourse.bass_utils import run_bass_kernel_spmd

# ---------------------------------------------------------------- problem dims
B = 1
M = 2048
H = 16
N = 4096
D = 128
N_CORES = 8
MS = M // N_CORES          # 256 rows of m per core
MT = MS // 128             # 2 m-tiles per core
FD = 1024                  # n-chunk (free dim) per epilogue op = 2 PSUM banks
NCH = N // FD              # 4 n-chunks
DVE_HEADS = int(os.environ.get("IDX_DVE_HEADS", "6"))   # fused-chain heads on VectorE
WARMUP_MMS = int(os.environ.get("IDX_WARMUP_MMS", "5"))   # dummy MMs to trip HAM warm
GPS_COMBINE = bool(int(os.environ.get("IDX_GPS_COMBINE", "0")))  # alternate combine DVE/GpSimd
# (measured slower: GpSimd's ~2.4us combine lands on the unit's critical finish)
DMA_ADDS = int(os.environ.get("IDX_DMA_ADDS", "3"))     # lvl0 pair-adds on DMA rings
GPS_ADDS = int(os.environ.get("IDX_GPS_ADDS", "2"))     # lvl0 pair-adds on GpSimd
DMA_FOLD = bool(int(os.environ.get("IDX_DMA_FOLD", "0")))  # one early lvl-1 fold on the rings
# (fold measured slower on HW: the extra SWDGE issue makes GpSimd the pacer)
PSA3 = bool(int(os.environ.get("IDX_PSA3", "0")))       # psA 3-deep / psV 1-deep PSUM split
MM_W = 512                                              # matmul moving width (1 PSUM bank)

BF16 = mybir.dt.bfloat16
F32 = mybir.dt.float32
# match the reference's bf16 rounding of SOFTMAX_SCALE
SCALE_BF16 = float(np.float32(np.array(D ** -0.5, dtype=ml_dtypes.bfloat16)))

# --------------------------------------------------- custom fused DVE op
# out = relu(in0 * s0) + in1   (s0 per-partition scalar [P,1])
import concourse.dve_ops as dve_ops
from concourse.dve_spec import Spec as _Spec, Src0 as _Src0, Src1 as _Src1, C0 as _C0
from concourse.dve_spec import relu as _relu, lower as _lower
from concourse.dve_uop import DveOpSpec as _DveOpSpec

_OP_NAME = "RELU_SCALE_ADD_ANT"


def _ref_relu_scale_add(in0, in1, s0, s1, imm2):
    x = np.nan_to_num(in0.astype(np.float32) * s0, nan=0.0, posinf=np.inf, neginf=-np.inf)
    return np.maximum(x, 0.0).astype(np.float32) + in1


def _register_relu_scale_add():
    for op in dve_ops.OPS:
        if op.name == _OP_NAME:
            return op
    spec = _Spec(body=_relu(_Src0 * _C0) + _Src1, reference=_ref_relu_scale_add)
    row = max(dve_ops._SUB_OPCODE_FOR_NAME.values()) + 1
    assert row < 0x20
    dve_ops._SUB_OPCODE_FOR_NAME[_OP_NAME] = row
    shas = {
        v: _DveOpSpec(name=_OP_NAME, opcode=row, uops=_lower(spec, ver=v), rd1_en=True).sha(v)
        for v in ("v3", "v4")
    }
    op = dve_ops.DveOp(_OP_NAME, spec, subdim=False, uops_sha=shas)
    dve_ops.OPS.append(op)
    dve_ops.CUSTOM_DVE_SPECS[_OP_NAME] = spec
    return op


RELU_SCALE_ADD = _register_relu_scale_add()

# Head roles per unit: ACT ("A") heads with the chain ("V") heads spread
# evenly among them (the baseline spread, measured best on HW).


def _head_roles(v_heads: int) -> list[str]:
    roles = ["A"] * H
    if v_heads > 0:
        step = H / v_heads
        for i in range(v_heads):
            roles[min(H - 1, int((i + 0.7) * step))] = "V"
    assert roles.count("V") == v_heads
    return roles


# ------------------------------------------------------------------ kernel IR
def _emit(ctx: ExitStack, tc: "tile.TileContext", q_d, k_d, o_d):
    nc = tc.nc
    AOp = mybir.AluOpType
    roles = _head_roles(DVE_HEADS)

    const = ctx.enter_context(tc.tile_pool(name="const", bufs=1))
    psA = ctx.enter_context(tc.tile_pool(name="psA", bufs=3 if PSA3 else 2, space="PSUM"))
    psV = ctx.enter_context(tc.tile_pool(name="psV", bufs=1 if PSA3 else 2, space="PSUM"))
    rpool = ctx.enter_context(tc.tile_pool(name="rpool", bufs=40))
    tpool = ctx.enter_context(tc.tile_pool(name="tpool", bufs=12))
    apool = ctx.enter_context(tc.tile_pool(name="apool", bufs=6))
    opool = ctx.enter_context(tc.tile_pool(name="opool", bufs=5))

    # ---- t=0: dummies. A bf16 zero tile (memset on VectorE — a GpSimd
    # memset would trigger a ~6us MODIFY_POOL_CONFIG IRAM load) feeds a
    # burst of matmuls that trips the PE HAM into the warm (2.4 GHz) state
    # while the input transposes are still in flight. The warmup PSUM tile
    # borrows a psA pool slot (PSUM is exactly full otherwise).
    dummy = const.tile([128, 512], BF16)
    nc.vector.memset(dummy[:], 0.0)
    if WARMUP_MMS:
        wu_ps = psA.tile([128, FD], F32, tag="logits", name="wu_ps")
        for i in range(WARMUP_MMS):
            nc.tensor.matmul(
                wu_ps[:, 0:512], dummy[:, 0:128], dummy[:], start=True, stop=True
            )

    # ---- input loads (plain 2D DMA — q is pre-scaled AND pre-transposed on
    # the host, k pre-transposed), split into pieces across BOTH HWDGE rings
    # (sync + scalar queues). qT cols are m-tile-major (col = mt*H*128 +
    # h*128 + m_local) so qT piece 0 covers every head of m-tile 0 -> unit 0
    # only needs [qT piece 0, kT piece 0]. A 1-col dummy ACTIVATE between
    # the scalar-queue loads forces the relu ACT-table load before the
    # first real eviction.
    kT = const.tile([128, N], BF16)
    qT = const.tile([128, H * MS], BF16)          # columns: mt*H*128 + h*128 + m
    QP = H * 128                                  # one m-tile's worth of q cols
    HQP = QP // 2
    nc.sync.dma_start(out=qT[:, 0:HQP], in_=q_d[:, 0:HQP])
    nc.scalar.dma_start(out=kT[:, 0:FD], in_=k_d[:, 0:FD])
    nc.sync.dma_start(out=qT[:, HQP:QP], in_=q_d[:, HQP:QP])
    d_act = const.tile([128, 1], BF16)
    nc.scalar.activation(d_act[:], dummy[:, 0:1], mybir.ActivationFunctionType.Relu)
    nc.sync.dma_start(out=kT[:, FD:2 * FD], in_=k_d[:, FD:2 * FD])
    nc.scalar.dma_start(out=kT[:, 2 * FD:3 * FD], in_=k_d[:, 2 * FD:3 * FD])
    nc.sync.dma_start(out=kT[:, 3 * FD:N], in_=k_d[:, 3 * FD:N])
    nc.sync.dma_start(out=qT[:, QP:2 * QP], in_=q_d[:, QP:2 * QP])

    # per-partition ones for the custom chain op's scale operand (the real
    # scale is folded into q on the host; weights >= 0 commutes with relu)
    ones = const.tile([128, 1], F32)
    nc.vector.memset(ones[:], 1.0)

    for mt in range(MT):
        for nci in range(NCH):
            n0 = nci * FD
            uid = f"{mt}_{nci}"
            # last unit: chain heads first (the ~1.3us/op chain must not
            # outlive the ACT stream) and late tree pairs off the DMA rings
            # (their completion latency would serialize into the kernel tail)
            local_tree = (mt == MT - 1) and (nci == NCH - 1)
            u_roles = roles
            if local_tree:
                u_roles = sorted(roles, key=lambda r: r != "V")
            acc = apool.tile([128, FD], F32, tag="acc", name=f"acc_{uid}")
            stage = opool.tile([128, FD], BF16, tag="stage", name=f"stage_{uid}")
            r_tiles = []      # bf16 ACT-evicted tiles awaiting tree
            dma_roots = []    # tiles holding in-place DMA pair sums
            gps_t = []        # GpSimd pair-sum tiles
            chain_i = 0
            prev = None       # chain accumulator AP (None until first V head)

            def _mk_head(h):
                pool = psV if u_roles[h] == "V" else psA
                pt = pool.tile([128, FD], F32, tag="logits", name=f"ps_{uid}_{h}")
                lhs = qT[:, mt * QP + h * 128: mt * QP + h * 128 + 128]
                for j in range(FD // MM_W):
                    nc.tensor.matmul(
                        pt[:, j * MM_W: (j + 1) * MM_W],
                        lhs,
                        kT[:, n0 + j * MM_W: n0 + (j + 1) * MM_W],
                        start=True,
                        stop=True,
                    )
                return pt

            def _emit_a(h):
                nonlocal r_tiles
                pt = _mk_head(h)
                r = rpool.tile([128, FD], BF16, tag="r", name=f"r_{uid}_{h}")
                nc.scalar.activation(
                    r[:], pt[:], mybir.ActivationFunctionType.Relu
                )
                r_tiles.append(r)
                # pair tiles up as they land: first pairs in-place on the
                # DMA rings, then GpSimd. The last unit alternates GpSimd /
                # VectorE adds instead so no DMA completion latency lands in
                # the kernel tail.
                tree_n = len(r_tiles)
                if tree_n >= 2 and tree_n % 2 == 0:
                    a, b = r_tiles[-2], r_tiles[-1]
                    pair_i = tree_n // 2 - 1
                    if local_tree and pair_i >= 2:
                        t = tpool.tile(
                            [128, FD], BF16, tag="t", name=f"t{len(gps_t)}_{uid}"
                        )
                        eng = nc.vector if pair_i % 2 else nc.gpsimd
                        eng.tensor_add(t[:], a[:], b[:])
                        gps_t.append(t)
                    elif pair_i < DMA_ADDS:
                        nc.gpsimd.dma_start(out=a[:], in_=b[:], accum_op=AOp.add)
                        dma_roots.append(a)
                        if DMA_FOLD and len(dma_roots) == 2 and pair_i == 1:
                            # one early level-1 fold (pairs 0+1 complete by
                            # mid-unit; a single non-chained fold stays off
                            # the unit's critical tail)
                            nc.gpsimd.dma_start(
                                out=dma_roots[0][:], in_=dma_roots[1][:],
                                accum_op=AOp.add,
                            )
                            dma_roots.pop()
                    elif pair_i < DMA_ADDS + GPS_ADDS:
                        t = tpool.tile(
                            [128, FD], BF16, tag="t", name=f"t{len(gps_t)}_{uid}"
                        )
                        nc.gpsimd.tensor_add(t[:], a[:], b[:])
                        gps_t.append(t)
                    else:
                        t = tpool.tile(
                            [128, FD], BF16, tag="t", name=f"t{len(gps_t)}_{uid}"
                        )
                        nc.vector.tensor_add(t[:], a[:], b[:])
                        gps_t.append(t)

            def _emit_v(h, in1, out_ap):
                # out = relu(psum) + in1   (in1 None -> plain relu)
                pt = _mk_head(h)
                if in1 is None:
                    nc.vector.tensor_scalar(
                        out_ap[:], pt[:], 1.0, 0.0, op0=AOp.mult, op1=AOp.max
                    )
                else:
                    nc.vector._custom_dve(
                        RELU_SCALE_ADD, out=out_ap[:], in0=pt[:], in1=in1[:],
                        s0=ones[:, 0:1],
                    )

            for h, role in enumerate(u_roles):
                if role == "A":
                    _emit_a(h)
                else:
                    _emit_v(h, prev, acc)
                    prev = acc
                    chain_i += 1

            # finish the ACT-side tree on VectorE (bf16 2x): balanced merge
            # of the DMA / GpSimd pair sums plus any unpaired leaf.
            work = dma_roots + gps_t
            if len(r_tiles) % 2:
                work.append(r_tiles[-1])
            wi = 0
            while len(work) > 1:
                nxt = []
                for i in range(0, len(work) - 1, 2):
                    t3 = tpool.tile([128, FD], BF16, tag="t", name=f"tu{wi}_{uid}")
                    wi += 1
                    nc.vector.tensor_add(t3[:], work[i][:], work[i + 1][:])
                    nxt.append(t3)
                if len(work) % 2:
                    nxt.append(work[-1])
                work = nxt
            root = work[0] if work else None

            if chain_i and root is not None:
                # alternate the combine between GpSimd and VectorE: both
                # queues run near-saturated and this splits the 1.2-2.4us
                # op across them (last unit stays on the faster VectorE)
                gps_c = GPS_COMBINE and not local_tree and nci % 2 == 0
                eng = nc.gpsimd if gps_c else nc.vector
                eng.tensor_add(stage[:], acc[:], root[:])
            elif chain_i:
                nc.vector.tensor_copy(stage[:], acc[:])
            else:
                nc.vector.tensor_copy(stage[:], root[:])
            nc.sync.dma_start(
                out=o_d[mt * 128: (mt + 1) * 128, n0: n0 + FD], in_=stage[:]
            )


_NC_CACHE = None


def _build():
    global _NC_CACHE
    if _NC_CACHE is not None:
        return _NC_CACHE
    nc = bacc.Bacc(
        "TRN2",
        target_bir_lowering=False,
        debug=False,
        enable_asserts=False,
        num_devices=N_CORES,
    )
    q_d = nc.dram_tensor("qT", [D, H * MS], BF16, kind="ExternalInput").ap()
    k_d = nc.dram_tensor("kT", [D, N], BF16, kind="ExternalInput").ap()
    o_d = nc.dram_tensor("o", [MS, N], BF16, kind="ExternalOutput").ap()
    with tile.TileContext(nc) as tc:
        with ExitStack() as ctx:
            _emit(ctx, tc, q_d, k_d, o_d)
    nc.compile()
    _NC_CACHE = (nc, q_d, k_d, o_d)
    return _NC_CACHE


def _shard_inputs(q, k, weights):
    bf16 = ml_dtypes.bfloat16
    q = np.asarray(q).astype(bf16, copy=False).reshape(M, H, D)
    k = np.asarray(k).astype(bf16, copy=False).reshape(N, D)
    w = np.asarray(weights).astype(bf16, copy=False).reshape(H, M)
    # q_s matches the reference's bf16 rounding: bf16(w) * bf16(scale)
    q_s = (w.astype(np.float32) * np.float32(SCALE_BF16)).astype(bf16)
    # fold the scale into q on the host (weights >= 0 commutes with relu)
    q_scaled = (q.astype(np.float32) * q_s.T[:, :, None].astype(np.float32)).astype(bf16)
    kT = np.ascontiguousarray(k.T)                      # [D, N]
    in_maps = []
    for c in range(N_CORES):
        m0 = c * MS
        # cols ordered m-tile-major: col = mt*H*128 + h*128 + m_local
        q_c = q_scaled[m0: m0 + MS].reshape(MT, 128, H, D).transpose(0, 2, 1, 3)
        qT_c = np.ascontiguousarray(q_c.reshape(MT * H * 128, D).T)
        in_maps.append({"qT": qT_c, "kT": kT})
    return in_maps


LAST_RESULTS = None


def kernel(q, k, weights):
    global LAST_RESULTS
    nc, *_ = _build()
    in_maps = _shard_inputs(q, k, weights)
    trace = bool(int(os.environ.get("IDX_TRACE", "0")))
    res = run_bass_kernel_spmd(
        nc, in_maps, core_ids=list(range(N_CORES)), trace=trace
    )
    LAST_RESULTS = res
    out = np.empty((B, M, N), np.float32)
    for c in range(N_CORES):
        out[0, c * MS: (c + 1) * MS] = res.results[c]["o"].astype(np.float32)
    return out



# revision 37
# speedup vs baseline: 1.3104x; 1.0059x over previous
"""Trainium2 Bass kernel for BF16IndexerBaseline (sparse_attention).

Computes, for q:(1,M,H,D) bf16, k:(1,N,D) bf16, weights:(H,M) bf16:

    index_score[b,m,n] = sum_h relu(q[b,m,h,:] . k[b,n,:]) * (weights[h,m]*D**-0.5)

Strategy (8 NeuronCores, SPMD, host-side sharding of m):
  - each core gets m-shard of 256 rows (2 m-tiles of 128), k replicated.
  - since weights >= 0, the per-(m,h) scale commutes with relu and is folded
    into the PSUM-eviction ops as a per-partition scalar (logits come out of
    the PE with m on partitions).
  - per (m-tile, n-chunk of 1024) unit: 16 heads x 2 matmuls (K=D=128
    contraction, stationary qT tile, moving kT) -> fp32 logits in PSUM
    ([128,1024] tiles, separate 2-buf pools for the A- and V-head roles).
  - epilogue split across engines (PSUM reads are the hard bottleneck:
    1 elem/lane/cyc per engine, ACT+DVE only):
      * 6 "chain" heads on VectorE via a runtime-registered fused custom
        DVE op RELU_SCALE_ADD: acc = relu(psum*s) + acc (fp32, 1 op/elem).
        The chain is kept independent of the ACT stream (no cross-seeding
        - coupling the two streams convoys the whole pipeline).
      * 10 heads on ScalarE: r = relu(psum*s) -> bf16 tiles; pair-summed
        as they land: 3 pairs on the DMA rings (SWDGE CCE accumulate,
        in-place SBUF->SBUF, no serial folds), 2 on GpSimd; balanced
        upper merge + final combine acc+root on VectorE (bf16 2x).
      * last unit: chain heads emitted first and late pairs on GpSimd /
        VectorE instead of the rings, so no chain overhang or DMA
        completion latency serializes into the kernel tail.
  - PE warm-up: a burst of dummy matmuls at t=0 keeps the PE HAM activity
    monitor busy through the input-transpose phase so real matmuls run at
    2.4 GHz instead of the cold 1.2 GHz.
  - startup: q sharded m-tile-major so unit 0 needs only the first q piece;
    q/k transposed through the DMA xbar in small pieces alternating across
    the two HWDGE rings (sync + scalar), critical pieces first; a dummy
    ACTIVATE preloads the relu table set before the transposes finish.
  - final: out = chain_acc + tree_root (fp32) -> DMA to DRAM.

Measured on 8x trn2 (NTFF profile): 124.7-126.0 us across runs in the
device's normal clock state (baseline 133.1 us); ~140-150 us when the
chip is in its P0 power-derate state (all engines ~15-20% slower —
device-side, affects any kernel equally). The
structure is pinned by hard limits: PSUM (16 KB/partition) holds exactly
4x [128,1024] fp32 tiles (2-buf ping-pong for each of the A/V roles, MM
output <= 1 bank = 512 fp32); eviction floor = 16.8M fp32 PSUM reads
through ACT (1.2 GHz) + DVE (0.96 GHz) at 1 elem/lane/cyc ~= 61 us + per
-op overheads (~(FD+282)/1.2 ns ACT, ~(FD+207)/0.96 ns DVE) ~= 87 us;
plus ~6.5 us framework preamble and ~6 us teardown barriers.
"""

import os

os.environ.setdefault("MYCRO_LOCAL_CACHE", "1")

import numpy as np
import ml_dtypes
from contextlib import ExitStack

import concourse.bass as bass
import concourse.tile as tile
from concourse import bacc, mybir
from concourse.bass_utils import run_bass_kernel_spmd

# ---------------------------------------------------------------- problem dims
B = 1
M = 2048
H = 16
N = 4096
D = 128
N_CORES = 8
MS = M // N_CORES          # 256 rows of m per core
MT = MS // 128             # 2 m-tiles per core
FD = 1024                  # n-chunk (free dim) per epilogue op = 2 PSUM banks
NCH = N // FD              # 4 n-chunks
DVE_HEADS = int(os.environ.get("IDX_DVE_HEADS", "6"))   # fused-chain heads on VectorE
WARMUP_MMS = int(os.environ.get("IDX_WARMUP_MMS", "5"))   # dummy MMs to trip HAM warm
GPS_COMBINE = bool(int(os.environ.get("IDX_GPS_COMBINE", "0")))  # alternate combine DVE/GpSimd
# (measured slower: GpSimd's ~2.4us combine lands on the unit's critical finish)
DMA_ADDS = int(os.environ.get("IDX_DMA_ADDS", "3"))     # lvl0 pair-adds on DMA rings
GPS_ADDS = int(os.environ.get("IDX_GPS_ADDS", "2"))     # lvl0 pair-adds on GpSimd
DMA_FOLD = bool(int(os.environ.get("IDX_DMA_FOLD", "0")))  # one early lvl-1 fold on the rings
# (fold measured slower on HW: the extra SWDGE issue makes GpSimd the pacer)
PSA3 = bool(int(os.environ.get("IDX_PSA3", "0")))       # psA 3-deep / psV 1-deep PSUM split
MM_W = 512                                              # matmul moving width (1 PSUM bank)

BF16 = mybir.dt.bfloat16
F32 = mybir.dt.float32
# match the reference's bf16 rounding of SOFTMAX_SCALE
SCALE_BF16 = float(np.float32(np.array(D ** -0.5, dtype=ml_dtypes.bfloat16)))

# --------------------------------------------------- custom fused DVE op
# out = relu(in0 * s0) + in1   (s0 per-partition scalar [P,1])
import concourse.dve_ops as dve_ops
from concourse.dve_spec import Spec as _Spec, Src0 as _Src0, Src1 as _Src1, C0 as _C0
from concourse.dve_spec import relu as _relu, lower as _lower
from concourse.dve_uop import DveOpSpec as _DveOpSpec

_OP_NAME = "RELU_SCALE_ADD_ANT"


def _ref_relu_scale_add(in0, in1, s0, s1, imm2):
    x = np.nan_to_num(in0.astype(np.float32) * s0, nan=0.0, posinf=np.inf, neginf=-np.inf)
    return np.maximum(x, 0.0).astype(np.float32) + in1


def _register_relu_scale_add():
    for op in dve_ops.OPS:
        if op.name == _OP_NAME:
            return op
    spec = _Spec(body=_relu(_Src0 * _C0) + _Src1, reference=_ref_relu_scale_add)
    row = max(dve_ops._SUB_OPCODE_FOR_NAME.values()) + 1
    assert row < 0x20
    dve_ops._SUB_OPCODE_FOR_NAME[_OP_NAME] = row
    shas = {
        v: _DveOpSpec(name=_OP_NAME, opcode=row, uops=_lower(spec, ver=v), rd1_en=True).sha(v)
        for v in ("v3", "v4")
    }
    op = dve_ops.DveOp(_OP_NAME, spec, subdim=False, uops_sha=shas)
    dve_ops.OPS.append(op)
    dve_ops.CUSTOM_DVE_SPECS[_OP_NAME] = spec
    return op


RELU_SCALE_ADD = _register_relu_scale_add()

# Head roles per unit: ACT ("A") heads with the chain ("V") heads spread
# evenly among them (the baseline spread, measured best on HW).


def _head_roles(v_heads: int) -> list[str]:
    roles = ["A"] * H
    if v_heads > 0:
        step = H / v_heads
        for i in range(v_heads):
            roles[min(H - 1, int((i + 0.7) * step))] = "V"
    assert roles.count("V") == v_heads
    return roles


# ------------------------------------------------------------------ kernel IR
def _emit(ctx: ExitStack, tc: "tile.TileContext", q_d, k_d, o_d):
    nc = tc.nc
    AOp = mybir.AluOpType
    roles = _head_roles(DVE_HEADS)

    const = ctx.enter_context(tc.tile_pool(name="const", bufs=1))
    psA = ctx.enter_context(tc.tile_pool(name="psA", bufs=3 if PSA3 else 2, space="PSUM"))
    psV = ctx.enter_context(tc.tile_pool(name="psV", bufs=1 if PSA3 else 2, space="PSUM"))
    rpool = ctx.enter_context(tc.tile_pool(name="rpool", bufs=40))
    tpool = ctx.enter_context(tc.tile_pool(name="tpool", bufs=12))
    apool = ctx.enter_context(tc.tile_pool(name="apool", bufs=6))
    opool = ctx.enter_context(tc.tile_pool(name="opool", bufs=5))

    # ---- t=0: dummies. A bf16 zero tile (memset on VectorE — a GpSimd
    # memset would trigger a ~6us MODIFY_POOL_CONFIG IRAM load) feeds a
    # burst of matmuls that trips the PE HAM into the warm (2.4 GHz) state
    # while the input transposes are still in flight. The warmup PSUM tile
    # borrows a psA pool slot (PSUM is exactly full otherwise).
    dummy = const.tile([128, 512], BF16)
    nc.vector.memset(dummy[:], 0.0)
    if WARMUP_MMS:
        wu_ps = psA.tile([128, FD], F32, tag="logits", name="wu_ps")
        for i in range(WARMUP_MMS):
            nc.tensor.matmul(
                wu_ps[:, 0:512], dummy[:, 0:128], dummy[:], start=True, stop=True
            )

    # ---- input loads (plain 2D DMA — q is pre-scaled AND pre-transposed on
    # the host, k pre-transposed), split into pieces across BOTH HWDGE rings
    # (sync + scalar queues). qT cols are m-tile-major (col = mt*H*128 +
    # h*128 + m_local) so qT piece 0 covers every head of m-tile 0 -> unit 0
    # only needs [qT piece 0, kT piece 0]. A 1-col dummy ACTIVATE between
    # the scalar-queue loads forces the relu ACT-table load before the
    # first real eviction.
    kT = const.tile([128, N], BF16)
    qT = const.tile([128, H * MS], BF16)          # columns: mt*H*128 + h*128 + m
    QP = H * 128                                  # one m-tile's worth of q cols
    HQP = QP // 2
    nc.sync.dma_start(out=qT[:, 0:HQP], in_=q_d[:, 0:HQP])
    nc.scalar.dma_start(out=kT[:, 0:FD], in_=k_d[:, 0:FD])
    nc.sync.dma_start(out=qT[:, HQP:QP], in_=q_d[:, HQP:QP])
    d_act = const.tile([128, 1], BF16)
    nc.scalar.activation(d_act[:], dummy[:, 0:1], mybir.ActivationFunctionType.Relu)
    nc.sync.dma_start(out=kT[:, FD:2 * FD], in_=k_d[:, FD:2 * FD])
    nc.scalar.dma_start(out=kT[:, 2 * FD:3 * FD], in_=k_d[:, 2 * FD:3 * FD])
    nc.sync.dma_start(out=kT[:, 3 * FD:N], in_=k_d[:, 3 * FD:N])
    nc.sync.dma_start(out=qT[:, QP:2 * QP], in_=q_d[:, QP:2 * QP])

    # per-partition ones for the custom chain op's scale operand (the real
    # scale is folded into q on the host; weights >= 0 commutes with relu)
    ones = const.tile([128, 1], F32)
    nc.vector.memset(ones[:], 1.0)

    for mt in range(MT):
        for nci in range(NCH):
            n0 = nci * FD
            uid = f"{mt}_{nci}"
            # last unit: chain heads first (the ~1.3us/op chain must not
            # outlive the ACT stream) and late tree pairs off the DMA rings
            # (their completion latency would serialize into the kernel tail)
            local_tree = (mt == MT - 1) and (nci == NCH - 1)
            u_roles = roles
            if local_tree:
                # chains first; last two evictions on VectorE ("D") so the
                # ScalarE stream ends ~2.2us earlier and the final pair can
                # fold right after, off the ring/GpSimd latency path.
                u_roles = sorted(roles, key=lambda r: r != "V")
                u_roles[-2:] = ["D", "D"]
            acc = apool.tile([128, FD], F32, tag="acc", name=f"acc_{uid}")
            stage = opool.tile([128, FD], BF16, tag="stage", name=f"stage_{uid}")
            r_tiles = []      # bf16 ACT-evicted tiles awaiting tree
            dma_roots = []    # tiles holding in-place DMA pair sums
            gps_t = []        # GpSimd pair-sum tiles
            chain_i = 0
            prev = None       # chain accumulator AP (None until first V head)

            def _mk_head(h):
                pool = psV if u_roles[h] == "V" else psA
                pt = pool.tile([128, FD], F32, tag="logits", name=f"ps_{uid}_{h}")
                lhs = qT[:, mt * QP + h * 128: mt * QP + h * 128 + 128]
                for j in range(FD // MM_W):
                    nc.tensor.matmul(
                        pt[:, j * MM_W: (j + 1) * MM_W],
                        lhs,
                        kT[:, n0 + j * MM_W: n0 + (j + 1) * MM_W],
                        start=True,
                        stop=True,
                    )
                return pt

            def _emit_a(h, on_dve=False):
                nonlocal r_tiles
                pt = _mk_head(h)
                r = rpool.tile([128, FD], BF16, tag="r", name=f"r_{uid}_{h}")
                if on_dve:
                    nc.vector.tensor_scalar(
                        r[:], pt[:], 1.0, 0.0, op0=AOp.mult, op1=AOp.max
                    )
                else:
                    nc.scalar.activation(
                        r[:], pt[:], mybir.ActivationFunctionType.Relu
                    )
                r_tiles.append(r)
                # pair tiles up as they land: first pairs in-place on the
                # DMA rings, then GpSimd. The last unit alternates GpSimd /
                # VectorE adds instead so no DMA completion latency lands in
                # the kernel tail.
                tree_n = len(r_tiles)
                if tree_n >= 2 and tree_n % 2 == 0:
                    a, b = r_tiles[-2], r_tiles[-1]
                    pair_i = tree_n // 2 - 1
                    if local_tree and pair_i >= 2:
                        t = tpool.tile(
                            [128, FD], BF16, tag="t", name=f"t{len(gps_t)}_{uid}"
                        )
                        eng = nc.gpsimd if pair_i % 2 else nc.vector
                        eng.tensor_add(t[:], a[:], b[:])
                        gps_t.append(t)
                    elif pair_i < DMA_ADDS:
                        nc.gpsimd.dma_start(out=a[:], in_=b[:], accum_op=AOp.add)
                        dma_roots.append(a)
                        if DMA_FOLD and len(dma_roots) == 2 and pair_i == 1:
                            # one early level-1 fold (pairs 0+1 complete by
                            # mid-unit; a single non-chained fold stays off
                            # the unit's critical tail)
                            nc.gpsimd.dma_start(
                                out=dma_roots[0][:], in_=dma_roots[1][:],
                                accum_op=AOp.add,
                            )
                            dma_roots.pop()
                    elif pair_i < DMA_ADDS + GPS_ADDS:
                        t = tpool.tile(
                            [128, FD], BF16, tag="t", name=f"t{len(gps_t)}_{uid}"
                        )
                        nc.gpsimd.tensor_add(t[:], a[:], b[:])
                        gps_t.append(t)
                    else:
                        t = tpool.tile(
                            [128, FD], BF16, tag="t", name=f"t{len(gps_t)}_{uid}"
                        )
                        nc.vector.tensor_add(t[:], a[:], b[:])
                        gps_t.append(t)

            def _emit_v(h, in1, out_ap):
                # out = relu(psum) + in1   (in1 None -> plain relu)
                pt = _mk_head(h)
                if in1 is None:
                    nc.vector.tensor_scalar(
                        out_ap[:], pt[:], 1.0, 0.0, op0=AOp.mult, op1=AOp.max
                    )
                else:
                    nc.vector._custom_dve(
                        RELU_SCALE_ADD, out=out_ap[:], in0=pt[:], in1=in1[:],
                        s0=ones[:, 0:1],
                    )

            for h, role in enumerate(u_roles):
                if role == "A":
                    _emit_a(h)
                elif role == "D":
                    _emit_a(h, on_dve=True)
                else:
                    _emit_v(h, prev, acc)
                    prev = acc
                    chain_i += 1

            # finish the ACT-side tree on VectorE (bf16 2x): balanced merge
            # of the DMA / GpSimd pair sums plus any unpaired leaf.
            work = dma_roots + gps_t
            if len(r_tiles) % 2:
                work.append(r_tiles[-1])
            wi = 0
            while len(work) > 1:
                nxt = []
                for i in range(0, len(work) - 1, 2):
                    t3 = tpool.tile([128, FD], BF16, tag="t", name=f"tu{wi}_{uid}")
                    wi += 1
                    nc.vector.tensor_add(t3[:], work[i][:], work[i + 1][:])
                    nxt.append(t3)
                if len(work) % 2:
                    nxt.append(work[-1])
                work = nxt
            root = work[0] if work else None

            if chain_i and root is not None:
                # alternate the combine between GpSimd and VectorE: both
                # queues run near-saturated and this splits the 1.2-2.4us
                # op across them (last unit stays on the faster VectorE)
                gps_c = GPS_COMBINE and not local_tree and nci % 2 == 0
                eng = nc.gpsimd if gps_c else nc.vector
                eng.tensor_add(stage[:], acc[:], root[:])
            elif chain_i:
                nc.vector.tensor_copy(stage[:], acc[:])
            else:
                nc.vector.tensor_copy(stage[:], root[:])
            nc.sync.dma_start(
                out=o_d[mt * 128: (mt + 1) * 128, n0: n0 + FD], in_=stage[:]
            )


_NC_CACHE = None


def _build():
    global _NC_CACHE
    if _NC_CACHE is not None:
        return _NC_CACHE
    nc = bacc.Bacc(
        "TRN2",
        target_bir_lowering=False,
        debug=False,
        enable_asserts=False,
        num_devices=N_CORES,
    )
    q_d = nc.dram_tensor("qT", [D, H * MS], BF16, kind="ExternalInput").ap()
    k_d = nc.dram_tensor("kT", [D, N], BF16, kind="ExternalInput").ap()
    o_d = nc.dram_tensor("o", [MS, N], BF16, kind="ExternalOutput").ap()
    with tile.TileContext(nc) as tc:
        with ExitStack() as ctx:
            _emit(ctx, tc, q_d, k_d, o_d)
    nc.compile()
    _NC_CACHE = (nc, q_d, k_d, o_d)
    return _NC_CACHE


def _shard_inputs(q, k, weights):
    bf16 = ml_dtypes.bfloat16
    q = np.asarray(q).astype(bf16, copy=False).reshape(M, H, D)
    k = np.asarray(k).astype(bf16, copy=False).reshape(N, D)
    w = np.asarray(weights).astype(bf16, copy=False).reshape(H, M)
    # q_s matches the reference's bf16 rounding: bf16(w) * bf16(scale)
    q_s = (w.astype(np.float32) * np.float32(SCALE_BF16)).astype(bf16)
    # fold the scale into q on the host (weights >= 0 commutes with relu)
    q_scaled = (q.astype(np.float32) * q_s.T[:, :, None].astype(np.float32)).astype(bf16)
    kT = np.ascontiguousarray(k.T)                      # [D, N]
    in_maps = []
    for c in range(N_CORES):
        m0 = c * MS
        # cols ordered m-tile-major: col = mt*H*128 + h*128 + m_local
        q_c = q_scaled[m0: m0 + MS].reshape(MT, 128, H, D).transpose(0, 2, 1, 3)
        qT_c = np.ascontiguousarray(q_c.reshape(MT * H * 128, D).T)
        in_maps.append({"qT": qT_c, "kT": kT})
    return in_maps


LAST_RESULTS = None


def kernel(q, k, weights):
    global LAST_RESULTS
    nc, *_ = _build()
    in_maps = _shard_inputs(q, k, weights)
    trace = bool(int(os.environ.get("IDX_TRACE", "0")))
    res = run_bass_kernel_spmd(
        nc, in_maps, core_ids=list(range(N_CORES)), trace=trace
    )
    LAST_RESULTS = res
    out = np.empty((B, M, N), np.float32)
    for c in range(N_CORES):
        out[0, c * MS: (c + 1) * MS] = res.results[c]["o"].astype(np.float32)
    return out

